# revision 19
# baseline (speedup 1.0000x reference)
"""LoRA-linear (dense fp32) on 8 Trainium2 NeuronCores.

out = x @ W_base.T + b_base + ((x @ A.T) @ B.T) * (alpha/r)

Full shapes: x [4, 2048, 4096] f32, W_base [4096, 4096], b_base [4096],
A [16, 4096], B [4096, 16]; out [4, 2048, 4096] f32.

Sharding: 4-way data-parallel over M = 4*2048 = 8192 flattened rows x
2-way tensor-parallel over out_features (4096 -> 2048 per group).
Core c handles m-rows [(c//2)*2048, ...) and out-cols [(c%2)*2048, ...).
A is replicated; b/B are sharded with out_features.

v5: host pre-arranges all operands into the exact bf16 SBUF tile
layouts (pure layout + precision prep; every matmul FLOP stays on
device), so the device kernel is DMA-in -> matmuls -> evict -> DMA-out.
  - All big loads ride ONE ring (sync HWDGE) in exact consumption
    order: x0 (kt-chunked), x1, W slab 0, x2, W slab 1, x3, W slabs
    2-3, x4..x15; at/bt ride the scalar ring.  A single queue drains
    FIFO across all 16 SDMA engines at full HBM rate, so the first
    things needed are the first things resident (launching everything
    at once diluted bandwidth so nothing completed until ~19us and
    the first matmul waited until 27.6us).
  - Output tiles are evicted by DVE and stored on the gpsimd SWDGE
    queue so they never queue behind loads.
  - Compute is emitted in "blocks": a block interleaves accumulation
    groups for the cartesian product m_list x slab_list, kt-major, so
    one LDWEIGHTS of xt[mi][kt] is shared by len(slab_list) matmuls.
    The K=17 lora+bias matmuls sit at each block's end (their xat
    dependency chain stays off the group-start critical path) and
    share one LDWEIGHTS of xat per mi.
  - Ramp: single-slab blocks widening from (m0,m1) to (m0,m1,m2) as
    x tiles land -- 4-6 matmuls per W chunk byte keeps the ramp's W
    demand under the ~358 GB/s HBM arrival rate, so the first
    m-tiles of compute cover the W arrival window; m2 catches up on
    slab 0 right after the ramp.  xa0/xa1 ride inside the first block
    (standalone xa would head-of-line-block the PE FIFO on x0's last
    chunk); xa for m-tile i+1 rides inside m-tile i's last block.
  - Last m-tile finishes with single-slab blocks so the closing
    evictions overlap the final matmuls.
"""

import numpy as np
import ml_dtypes

import concourse.bass as bass
import concourse.tile as tile
from concourse import bacc, mybir
from concourse import bass_utils
from concourse.bass import ts
from concourse.bass_interp import get_hw_module
from concourse.masks import make_identity

P = 128
D = 4096                 # in_features (contraction)
M_FULL = 8192            # 4 * 2048 flattened rows
O_FULL = 4096            # out_features
MGRID, OGRID = 4, 2      # core grid: 4 data-parallel x 2 tensor-parallel
M_SHARD = M_FULL // MGRID    # 2048
O_SHARD = O_FULL // OGRID    # 2048
KT = D // P              # 32 contraction tiles
MT = M_SHARD // P        # 16 m-tiles
OC = 512                 # psum free dim per output tile
NOC = O_SHARD // OC      # 4 output slabs
WCH = 8                  # kt per W DMA chunk (1MB chunks)
XCH = 8                  # kt per x0 ramp chunk (256KB chunks)
R = 16                   # lora rank
RB = R + 1               # lora rank + bias row
SCALING = 32.0 / 16.0    # alpha / r

F32 = mybir.dt.float32
BF16 = mybir.dt.bfloat16
BF16_NP = ml_dtypes.bfloat16

_NC_CACHE = None


def _build_nc():
    nc = bacc.Bacc("TRN2", target_bir_lowering=False, debug=False, num_devices=8)
    x_d = nc.dram_tensor("x_t", [MT * P, KT * P], BF16, kind="ExternalInput").ap()
    w_d = nc.dram_tensor("w_t", [P, NOC * KT * OC], BF16, kind="ExternalInput").ap()
    a_d = nc.dram_tensor("a_t", [P, KT * R], BF16, kind="ExternalInput").ap()
    bt_d = nc.dram_tensor("bt_s", [RB, O_SHARD], BF16, kind="ExternalInput").ap()
    out_d = nc.dram_tensor("out_s", [M_SHARD, O_SHARD], F32, kind="ExternalOutput").ap()

    with tile.TileContext(nc) as tc:
        with (
            tc.tile_pool(name="const", bufs=1) as const,
            tc.tile_pool(name="xtp", bufs=4) as xtp,
            tc.tile_pool(name="ostage", bufs=4) as ostage,
            tc.tile_pool(name="small", bufs=2) as small,
            tc.tile_pool(name="ps_out", bufs=6, space="PSUM") as ps_out,
            tc.tile_pool(name="ps_sm", bufs=2, space="PSUM") as ps_sm,
        ):
            ident = const.tile([P, P], F32)
            make_identity(nc, ident)

            # xa.T resident, rank rows 0..15 plus a ones row (bias lane).
            # Whole-tile memset; rows 0..15 are overwritten per m-tile.
            xat_sb = const.tile([RB, M_SHARD], BF16)
            nc.any.memset(xat_sb[:, :], 1.0)

            at_sb = const.tile([P, KT * R], BF16)
            bt_sb = const.tile([RB, O_SHARD], BF16)
            wt_sb = const.tile([P, NOC, KT, OC], BF16)
            xt_tiles = [None] * MT
            psxa_tiles = [None] * MT

            def emit_x(mi, eng=None, chunked=False):
                eng = eng or nc.sync
                xt = xtp.tile([P, KT, P], BF16, tag="xt", name=f"xt_{mi}")
                if chunked:
                    for kq in range(KT // XCH):
                        eng.dma_start(
                            xt[:, kq * XCH:(kq + 1) * XCH, :],
                            x_d[ts(mi, P), kq * XCH * P:(kq + 1) * XCH * P])
                else:
                    eng.dma_start(xt[:], x_d[ts(mi, P), :])
                xt_tiles[mi] = xt

            def emit_w_slab(s):
                for kq in range(KT // WCH):
                    lo = s * KT * OC + kq * WCH * OC
                    nc.sync.dma_start(
                        wt_sb[:, s, kq * WCH:(kq + 1) * WCH, :],
                        w_d[:, lo:lo + WCH * OC])

            def finish_xa(mi):
                # evict + transpose xa -> xat_sb rows 0..15
                xa_sb = small.tile([P, R], F32, tag="xa")
                nc.vector.tensor_copy(xa_sb[:], psxa_tiles[mi][:])
                psxat = ps_sm.tile([R, P], F32, tag="sm")
                nc.tensor.transpose(psxat[:], xa_sb[:], ident[:])
                nc.vector.tensor_copy(xat_sb[0:R, ts(mi, P)], psxat[:])

            def emit_block(mis, ss, xa_for=(), split_tail=False):
                # Interleaved accumulation groups for mis x ss.  Each
                # group: 32 k-tile matmuls + one K=17 matmul (lora
                # delta + bias).  kt-major with s inner so one LDW of
                # xt[mi][kt] feeds len(ss) MMs.  xa matmuls for the
                # m-tiles in xa_for ride between the main matmuls.
                psos = {}
                for mi in mis:
                    for s in ss:
                        psos[(mi, s)] = ps_out.tile(
                            [P, OC], F32, tag="out", name=f"pso_{mi}_{s}")
                for xmi in xa_for:
                    psxa_tiles[xmi] = ps_sm.tile(
                        [P, R], F32, tag="sm", name=f"psxa_{xmi}")
                for kt in range(KT):
                    for mi in mis:
                        for s in ss:
                            nc.tensor.matmul(
                                psos[(mi, s)][:], xt_tiles[mi][:, kt, :],
                                wt_sb[:, s, kt, :],
                                start=(kt == 0), stop=False)
                    for xmi in xa_for:
                        nc.tensor.matmul(
                            psxa_tiles[xmi][:], xt_tiles[xmi][:, kt, :],
                            at_sb[:, ts(kt, R)],
                            start=(kt == 0), stop=(kt == KT - 1))
                # finish_xa BEFORE the K17s: in the ramp the K17s of
                # this very block consume xat rows written here, and
                # this keeps the PE FIFO order consistent with the
                # dependency order.
                for xmi in xa_for:
                    finish_xa(xmi)
                # K=17 lora+bias matmuls last: keeps the xat dependency
                # (xa -> evict -> transpose -> copy chain) off the
                # group-start critical path; adjacent K17s share one
                # LDW of xat per mi.
                for mi in mis:
                    for s in ss:
                        nc.tensor.matmul(
                            psos[(mi, s)][:], xat_sb[:, ts(mi, P)],
                            bt_sb[:, ts(s, OC)], start=False, stop=True)
                # stores ride SWDGE (gpsimd) so they never queue behind
                # the load ring; the split tail goes on scalar (idle by
                # then, lower completion latency).
                for (mi, s), pso in psos.items():
                    nsplit = 2 if split_tail else 1
                    for h in range(nsplit):
                        w = OC // nsplit
                        ob = ostage.tile([P, w], F32, tag="ob")
                        nc.vector.tensor_copy(ob[:], pso[:, h * w:(h + 1) * w])
                        eng = nc.scalar if split_tail else nc.gpsimd
                        eng.dma_start(
                            out_d[ts(mi, P),
                                  s * OC + h * w:s * OC + (h + 1) * w],
                            ob[:])

            # ---- load rings, exact consumption order ----
            # sync ring carries everything big, in consumption order;
            # at/bt (tiny) ride the otherwise idle scalar ring.
            nc.scalar.dma_start(at_sb[:], a_d[:, :])
            nc.scalar.dma_start(bt_sb[:], bt_d[:, :])
            emit_x(0, chunked=True)
            emit_x(1)
            emit_w_slab(0)
            emit_x(2)
            emit_w_slab(1)
            emit_x(3)
            emit_w_slab(2)
            emit_w_slab(3)
            for mi in range(4, MT):
                emit_x(mi)

            # ---- compute ----
            # Ramp: single-slab blocks, widening from (m0,m1) to
            # (m0,m1,m2) as x tiles land -- 4-6 matmuls per W chunk
            # byte keeps the ramp's W demand under the ~358 GB/s HBM
            # rate, so the first m-tiles of compute cover the W
            # arrival window.  xa0/xa1 ride inside the first block
            # (standalone xa would head-of-line-block the PE FIFO on
            # x0's last chunk).  m2 skips slab 0 during the ramp and
            # catches up right after, when W is fully resident.
            emit_block([0, 1], [0], xa_for=(0, 1))
            emit_block([0, 1, 2], [1], xa_for=(2,))
            emit_block([0, 1, 2], [2], xa_for=(3,))
            emit_block([0, 1, 2], [3])
            emit_block([2], [0])
            for mi in range(3, MT - 1):
                emit_block([mi], [0, 1, 2, 3], xa_for=(mi + 1,))
            # last m-tile: single-slab final blocks so the closing
            # evictions overlap the last matmuls
            emit_block([MT - 1], [0, 1])
            emit_block([MT - 1], [2])
            emit_block([MT - 1], [3], split_tail=True)

    nc.compile()
    nc.m = get_hw_module(nc.m)
    return nc


def _get_nc():
    global _NC_CACHE
    if _NC_CACHE is None:
        _NC_CACHE = _build_nc()
    return _NC_CACHE


def _make_in_maps(x, W_base, b_base, A, B):
    bf = BF16_NP
    xf = np.asarray(x, np.float32).reshape(M_FULL, D)
    W = np.asarray(W_base, np.float32)
    b = np.asarray(b_base, np.float32)
    A = np.asarray(A, np.float32)
    Bm = np.asarray(B, np.float32)

    # A.T tiles: at[p, kt*R + r] = A[r, kt*128 + p]
    at = np.ascontiguousarray(
        A.reshape(R, KT, P).transpose(2, 1, 0)).reshape(P, KT * R).astype(bf)

    x_bf = xf.astype(bf)
    W_bf = W.astype(bf)

    xt_cache, w_cache, bt_cache = {}, {}, {}
    in_maps = []
    for c in range(MGRID * OGRID):
        i, j = divmod(c, OGRID)
        if i not in xt_cache:
            xs = x_bf[i * M_SHARD:(i + 1) * M_SHARD]
            # [mt, m, kt, p] -> [mt, p, kt, m]
            xt_cache[i] = np.ascontiguousarray(
                xs.reshape(MT, P, KT, P).transpose(0, 3, 2, 1)
            ).reshape(MT * P, KT * P)
        if j not in w_cache:
            Ws = W_bf[j * O_SHARD:(j + 1) * O_SHARD]
            # [s, o, kt, p] -> [p, s, kt, o]
            w_cache[j] = np.ascontiguousarray(
                Ws.reshape(NOC, OC, KT, P).transpose(3, 0, 2, 1)
            ).reshape(P, NOC * KT * OC)
            bt = np.empty((RB, O_SHARD), np.float32)
            bt[0:R] = SCALING * Bm[j * O_SHARD:(j + 1) * O_SHARD].T
            bt[R] = b[j * O_SHARD:(j + 1) * O_SHARD]
            bt_cache[j] = bt.astype(bf)
        in_maps.append({
            "x_t": xt_cache[i],
            "w_t": w_cache[j],
            "a_t": at,
            "bt_s": bt_cache[j],
        })
    return in_maps


def _gather(results):
    out = np.empty((M_FULL, O_FULL), np.float32)
    for c in range(MGRID * OGRID):
        i, j = divmod(c, OGRID)
        out[i * M_SHARD:(i + 1) * M_SHARD, j * O_SHARD:(j + 1) * O_SHARD] = \
            results[c]["out_s"]
    return out.reshape(4, 2048, 4096)


def run(x, W_base, b_base, A, B, trace=False, trace_kwargs=None):
    nc = _get_nc()
    in_maps = _make_in_maps(x, W_base, b_base, A, B)
    res = bass_utils.run_bass_kernel_spmd(
        nc, in_maps, core_ids=list(range(8)), trace=trace,
        **(trace_kwargs or {}),
    )
    return _gather(res.results), res


def kernel(x, W_base, b_base, A, B):
    out, _ = run(x, W_base, b_base, A, B, trace=False)
    return out
